# revision 1
# baseline (speedup 1.0000x reference)
# Trainium2 Bass kernel for BidirectionalCrossAttentionBlock.
#
# Key algebraic facts used (validated vs the reference to 1e-5):
#  * attn_i2t is a softmax over a size-1 axis -> identically 1.0, so
#    img_guided[b,c,n] = v_txt[b,c] (broadcast).
#  * The t2i attention, gating and out-projection collapse to rank-2
#    updates over the image:
#      out1 = x + wv (x) alpha + wt (x) beta + b_out
#    with per-position alpha[n] = img_imp/denom, beta[n] = txt_imp/denom,
#    wv = w_out@v_txt, wt = w_out@(w_img_v@s),
#    s[c] = g[c]*(sum_n u[n] x[c,n] - K1)/Z + b[c],
#    u[n] = exp(SCALE*logit[n])*rstd[n], Z = sum exp, K1 = sum u*m.
#  * logit[n] and the image gate are single matvecs against raw x plus
#    per-position corrections from the LN stats (m[n], rstd[n]).
#  * Only real heavy compute: the two FFN 1x1 convs; done as fp32r
#    matmuls (full PE rate at N=512) with the rank-2/bias terms folded
#    in as small extra-K matmuls.
#
# Sharding: pure data parallel, 2 batches per core on 8 cores.

import os
import numpy as np

import concourse.bacc as bacc
import concourse.tile as tile
import concourse.mybir as mybir
from concourse import bass_utils
from concourse.masks import make_identity

F32 = mybir.dt.float32
F32R = mybir.dt.float32r
AL = mybir.AluOpType
AF = mybir.ActivationFunctionType

B, C, H, W = 16, 256, 64, 64
GUIDE = 512
HW = 64 * 64  # 4096
NCORES = 8
BSH = B // NCORES  # 2 batches per core
SCALE = (C // 4) ** -0.5  # heads=4 -> 0.125
EPS_LN = 1e-5
EPS_FUSE = 1e-8
NBLK = 8          # 4096 / 512
BLK = 512
CT = 2            # channel tiles of 128
KT = GUIDE // 128  # 4

_CACHE = {}


def _build(debug=False):
    nc = bacc.Bacc("TRN2", target_bir_lowering=False, debug=False)

    # ---------------- DRAM tensors ----------------
    img = nc.dram_tensor("img", [BSH, C, HW], F32R, kind="ExternalInput")
    txt = nc.dram_tensor("txt", [BSH, GUIDE], F32, kind="ExternalInput")
    # weights (host pre-transposed where noted), all fp32r (matmul operands)
    wf1T = nc.dram_tensor("wf1T", [C, C], F32R, kind="ExternalInput")   # w_ffn1.T
    wf2T = nc.dram_tensor("wf2T", [C, C], F32R, kind="ExternalInput")   # w_ffn2.T
    woutT = nc.dram_tensor("woutT", [C, C], F32R, kind="ExternalInput")  # w_out.T
    wivT = nc.dram_tensor("wivT", [C, C], F32R, kind="ExternalInput")   # w_img_v.T
    wik = nc.dram_tensor("wik", [C, C], F32R, kind="ExternalInput")     # w_img_k
    wtqT = nc.dram_tensor("wtqT", [GUIDE, C], F32R, kind="ExternalInput")  # w_txt_q.T
    wtvT = nc.dram_tensor("wtvT", [GUIDE, C], F32R, kind="ExternalInput")  # w_txt_v.T
    # [C]-vectors as [128, 2] (col = ctile)
    g2d = nc.dram_tensor("g2d", [128, CT], F32, kind="ExternalInput")    # ln_img_g
    bln2d = nc.dram_tensor("bln2d", [128, CT], F32, kind="ExternalInput")  # ln_img_b
    bb2d = nc.dram_tensor("bb2d", [128, CT], F32R, kind="ExternalInput")   # b_out+b_ffn2
    cc12d = nc.dram_tensor("cc12d", [128, CT], F32, kind="ExternalInput")  # w_ffn1@b_out+b_ffn1
    wg2d = nc.dram_tensor("wg2d", [128, CT], F32R, kind="ExternalInput")   # w_igate*g
    # txt-side row vectors replicated over 2 partitions
    gt2d = nc.dram_tensor("gt2d", [1, GUIDE], F32, kind="ExternalInput")   # ln_txt_g
    bt2d = nc.dram_tensor("bt2d", [1, GUIDE], F32, kind="ExternalInput")   # ln_txt_b
    wtg2d = nc.dram_tensor("wtg2d", [1, GUIDE], F32, kind="ExternalInput")  # w_tgate
    btg2d = nc.dram_tensor("btg2d", [2, 1], F32, kind="ExternalInput")     # b_tgate repl
    misc = nc.dram_tensor("misc", [1, 8], F32, kind="ExternalInput")       # Sg, Cg, ...
    conesr = nc.dram_tensor("conesr", [1, 128], F32R, kind="ExternalInput")  # ones
    id128d = nc.dram_tensor("id128d", [128, 128], F32R, kind="ExternalInput")  # identity

    yout = nc.dram_tensor("yout", [BSH, C, HW], F32, kind="ExternalOutput")
    if debug:
        dbg_ext = nc.dram_tensor("dbg_ext", [BSH, 4, HW], F32, kind="ExternalOutput")
        dbg_s = nc.dram_tensor("dbg_s", [BSH, 128, CT], F32, kind="ExternalOutput")
        dbg_sc = nc.dram_tensor("dbg_sc", [BSH, 1, 8], F32, kind="ExternalOutput")

    env = {k: v for k, v in locals().items()}
    from contextlib import ExitStack
    with tile.TileContext(nc) as tc, ExitStack() as ctx:
        env["wp"] = ctx.enter_context(tc.tile_pool(name="wp", bufs=1))
        env["xp"] = ctx.enter_context(tc.tile_pool(name="xp", bufs=2))
        env["bigp"] = ctx.enter_context(tc.tile_pool(name="bigp", bufs=2))
        env["outp"] = ctx.enter_context(tc.tile_pool(name="outp", bufs=2))
        env["smp"] = ctx.enter_context(tc.tile_pool(name="smp", bufs=2))
        env["drp"] = ctx.enter_context(tc.tile_pool(name="drp", bufs=2, space="DRAM"))
        env["psC"] = ctx.enter_context(tc.tile_pool(name="psC", bufs=4, space="PSUM"))
        env["psA"] = ctx.enter_context(tc.tile_pool(name="psA", bufs=1, space="PSUM"))
        env["psB"] = ctx.enter_context(tc.tile_pool(name="psB", bufs=1, space="PSUM"))
        env["psM"] = ctx.enter_context(tc.tile_pool(name="psM", bufs=2, space="PSUM"))
        _emit(nc, tc, env, debug)
    nc.compile()
    return nc


def _emit(nc, tc, env, debug):
    STAGE = float(os.environ.get("KSTAGE", "9"))
    REPEAT = int(os.environ.get("KREPEAT", "1"))
    wp, xp, bigp, outp, smp, drp = (env[k] for k in ("wp", "xp", "bigp", "outp", "smp", "drp"))
    psC, psA, psB, psM = (env[k] for k in ("psC", "psA", "psB", "psM"))
    img, txt, yout = env["img"], env["txt"], env["yout"]

    ap = lambda name: env[name].ap()

    # ---------------- load weights/consts ----------------
    def load_w2(name):  # [C, C] -> [128, 2, C] (ktile along dim1)
        t = wp.tile([128, CT, C], F32R, tag=name)
        for kt in range(CT):
            nc.sync.dma_start(out=t[:, kt, :], in_=ap(name)[kt * 128:(kt + 1) * 128, :])
        return t

    def load_w4(name):  # [GUIDE, C] -> [128, 4, C]
        t = wp.tile([128, KT, C], F32R, tag=name)
        for kt in range(KT):
            nc.sync.dma_start(out=t[:, kt, :], in_=ap(name)[kt * 128:(kt + 1) * 128, :])
        return t

    wf1 = load_w2("wf1T")
    wf2 = load_w2("wf2T")
    wou = load_w2("woutT")
    wiv = load_w2("wivT")
    wik_s = load_w2("wik")
    wtq = load_w4("wtqT")
    wtv = load_w4("wtvT")

    def load_v(name, dt_):
        t = wp.tile([128, CT], dt_, tag=name)
        nc.sync.dma_start(out=t, in_=ap(name))
        return t

    g2 = load_v("g2d", F32)
    bln2 = load_v("bln2d", F32)
    bb2 = load_v("bb2d", F32R)
    cc12 = load_v("cc12d", F32)
    wg2 = load_v("wg2d", F32R)

    def load_r2(name):  # [1, GUIDE] -> [2, GUIDE] partition-broadcast
        t = wp.tile([2, GUIDE], F32, tag=name)
        src = ap(name)
        bc = src.__class__(tensor=src.tensor, offset=src.offset,
                           ap=[[0, 2]] + [list(d) for d in src.ap[1:]])
        nc.sync.dma_start(out=t, in_=bc)
        return t

    gt2 = load_r2("gt2d")
    bt2 = load_r2("bt2d")
    wtg2 = load_r2("wtg2d")
    btg2 = wp.tile([2, 1], F32, tag="btg2d")
    nc.sync.dma_start(out=btg2, in_=ap("btg2d"))
    misc_sb = wp.tile([1, 8], F32, tag="misc")
    nc.sync.dma_start(out=misc_sb, in_=ap("misc"))

    onesrow = wp.tile([1, 128], F32R, tag="onesrow")
    nc.sync.dma_start(out=onesrow, in_=ap("conesr"))
    onescol = wp.tile([128, 1], F32R, tag="onescol")
    nc.sync.dma_start(out=onescol, in_=ap("conesr").rearrange("o p -> p o"))
    ones32x = wp.tile([32, 128], F32R, tag="ones32x")
    src = ap("conesr")
    bc = src.__class__(tensor=src.tensor, offset=src.offset,
                       ap=[[0, 32]] + [list(d) for d in src.ap[1:]])
    nc.sync.dma_start(out=ones32x, in_=bc)
    id128 = wp.tile([128, 128], F32R, tag="id128")
    nc.sync.dma_start(out=id128, in_=ap("id128d"))
    id2 = wp.tile([2, 2], F32, tag="id2")
    make_identity(nc, id2[:])
    onescf = wp.tile([128, 1], F32, tag="onescf")
    nc.vector.memset(onescf[:], 1.0)
    eps2 = wp.tile([2, 1], F32, tag="eps2")
    nc.vector.memset(eps2[:], EPS_LN)
    eps32 = wp.tile([32, 1], F32, tag="eps32")
    nc.vector.memset(eps32[:], EPS_LN)

    if STAGE <= 0:
        return
    # ---------------- txt side (both batches) ----------------
    txt_sb = smp.tile([2, GUIDE], F32, tag="txt", bufs=1)
    nc.sync.dma_start(out=txt_sb, in_=txt.ap())
    bnst = smp.tile([2, 6], F32, tag="bnst")
    nc.vector.bn_stats(out=bnst[:], in_=txt_sb[:])
    mv = smp.tile([2, 2], F32, tag="mv")
    nc.vector.bn_aggr(out=mv[:], in_=bnst[:])
    tstd = smp.tile([2, 1], F32, tag="tstd")
    nc.scalar.activation(tstd[:], mv[:, 1:2], AF.Sqrt, bias=eps2[:], scale=1.0)
    trstd = smp.tile([2, 1], F32, tag="trstd")
    nc.vector.reciprocal(trstd[:], tstd[:])
    txtn = smp.tile([2, GUIDE], F32, tag="txtn", bufs=1)
    nc.vector.tensor_scalar(out=txtn[:], in0=txt_sb[:], scalar1=mv[:, 0:1],
                            scalar2=trstd[:], op0=AL.subtract, op1=AL.mult)
    nc.vector.tensor_mul(txtn[:], txtn[:], gt2[:])
    nc.vector.tensor_add(txtn[:], txtn[:], bt2[:])
    if STAGE <= 0.2:
        return
    # txt_imp = sigmoid(txtn @ w_tgate + b_tgate)  [2,1]
    scr2 = smp.tile([2, GUIDE], F32, tag="scr2", bufs=1)
    tip = smp.tile([2, 1], F32, tag="tip")
    nc.vector.tensor_mul(scr2[:], txtn[:], wtg2[:])
    nc.vector.reduce_sum(tip[:], scr2[:], axis=mybir.AxisListType.X)
    nc.vector.tensor_add(tip[:], tip[:], btg2[:])
    ti = smp.tile([2, 1], F32, tag="ti")
    nc.scalar.activation(ti[:], tip[:], AF.Sigmoid)
    if STAGE <= 0.4:
        return
    # transpose ti -> [1, 2]
    pst = psM.tile([1, 2], F32, tag="psm")
    nc.tensor.transpose(pst[:], ti[:], id2[:])
    tiT = smp.tile([1, 2], F32, tag="tiT")
    nc.vector.tensor_copy(tiT[:], pst[:])
    if STAGE <= 0.5:
        return
    # txtn -> transposed fp32r [128, 4, 2]
    txtnT = smp.tile([128, KT, 2], F32R, tag="txtnT")
    for kt in range(KT):
        ps2 = psM.tile([128, 2], F32, tag="psm")
        nc.tensor.transpose(ps2[:], txtn[:, kt * 128:(kt + 1) * 128], id2[:])
        nc.vector.tensor_copy(txtnT[:, kt, :], ps2[:])
    if STAGE <= 0.6:
        return
    # q = w_txt_q @ txt_n, v = w_txt_v @ txt_n   -> [128, mt, b] fp32r
    q_sb = smp.tile([128, CT, 2], F32R, tag="q_sb")
    v_sb = smp.tile([128, CT, 2], F32R, tag="v_sb")
    for (wmat, dst) in ((wtq, q_sb), (wtv, v_sb)):
        for mt in range(CT):
            psq = psM.tile([128, 2], F32, tag="psm")
            for kt in range(KT):
                nc.tensor.matmul(psq[:], wmat[:, kt, mt * 128:(mt + 1) * 128],
                                 txtnT[:, kt, :], start=(kt == 0), stop=(kt == KT - 1))
            nc.vector.tensor_copy(dst[:, mt, :], psq[:])
    if STAGE <= 0.7:
        return
    # qk = w_img_k.T @ q  -> [128, mt, b] (fp32 copy for DVE use)
    qk_sb = smp.tile([128, CT, 2], F32, tag="qk_sb")
    for mt in range(CT):
        psk = psM.tile([128, 2], F32, tag="psm")
        for kt in range(CT):
            nc.tensor.matmul(psk[:], wik_s[:, kt, mt * 128:(mt + 1) * 128],
                             q_sb[:, kt, :], start=(kt == 0), stop=(kt == CT - 1))
        nc.vector.tensor_copy(qk_sb[:, mt, :], psk[:])
    if STAGE <= 0.8:
        return
    # wv = w_out @ v -> [128, mt, b] fp32r ; A1 = w_ffn1 @ wv
    wv_sb = smp.tile([128, CT, 2], F32R, tag="wv_sb")
    for mt in range(CT):
        psv = psM.tile([128, 2], F32, tag="psm")
        for kt in range(CT):
            nc.tensor.matmul(psv[:], wou[:, kt, mt * 128:(mt + 1) * 128],
                             v_sb[:, kt, :], start=(kt == 0), stop=(kt == CT - 1))
        nc.vector.tensor_copy(wv_sb[:, mt, :], psv[:])
    a1_sb = smp.tile([128, CT, 2], F32R, tag="a1_sb")
    for mt in range(CT):
        psa = psM.tile([128, 2], F32, tag="psm")
        for kt in range(CT):
            nc.tensor.matmul(psa[:], wf1[:, kt, mt * 128:(mt + 1) * 128],
                             wv_sb[:, kt, :], start=(kt == 0), stop=(kt == CT - 1))
        nc.vector.tensor_copy(a1_sb[:, mt, :], psa[:])

    # ---------------- per-batch, phase-major ----------------
    def pass1(b):
        st_ = {}
        xb = []
        for ct in range(CT):
            t = xp.tile([128, HW], F32R, tag=f"x{ct}", name=f"x{ct}")
            nc.sync.dma_start(out=t, in_=img.ap()[b, ct * 128:(ct + 1) * 128, :])
            xb.append(t)
        st_["xb"] = xb
        # W3 lhsT = [ones, qg_b, wg]
        W3 = smp.tile([128, CT, 3], F32R, tag="W3", name="W3")
        for ct in range(CT):
            nc.vector.tensor_copy(W3[:, ct, 0:1], onescol[:])
            nc.vector.tensor_mul(W3[:, ct, 1:2], qk_sb[:, ct, b:b + 1], g2[:, ct:ct + 1])
            nc.vector.tensor_copy(W3[:, ct, 2:3], wg2[:, ct:ct + 1])
        # Sq = sum qg ; Cq = sum qk*b_ln
        ps1 = psM.tile([1, 2], F32, tag="psm", name="ps1")
        for ct in range(CT):
            nc.tensor.matmul(ps1[:], W3[:, ct, 1:2], onescol[:].to_broadcast((128, 2)),
                             start=(ct == 0), stop=(ct == CT - 1))
        tcq = smp.tile([128, CT], F32R, tag="tcq", name="tcq")
        for ct in range(CT):
            nc.vector.tensor_mul(tcq[:, ct:ct + 1], qk_sb[:, ct, b:b + 1], bln2[:, ct:ct + 1])
        ps1b = psM.tile([1, 2], F32, tag="psm", name="ps1b")
        for ct in range(CT):
            nc.tensor.matmul(ps1b[:], tcq[:, ct:ct + 1], onescol[:].to_broadcast((128, 2)),
                             start=(ct == 0), stop=(ct == CT - 1))
        scal = smp.tile([1, 8], F32R, tag="scal", name="scal")
        nc.vector.tensor_copy(scal[0:1, 0:1], ps1[:, 0:1])
        nc.vector.tensor_copy(scal[0:1, 1:2], ps1b[:, 0:1])
        nc.vector.tensor_copy(scal[0:1, 2:3], tiT[0:1, b:b + 1])
        nc.vector.tensor_copy(scal[0:1, 3:5], misc_sb[0:1, 0:2])
        ps32 = psM.tile([32, 8], F32, tag="psm", name="ps32")
        nc.tensor.matmul(ps32[:], onesrow[:, 0:32], scal[:], start=True, stop=True)
        sc32 = smp.tile([32, 8], F32, tag="sc32", name="sc32")
        nc.vector.tensor_copy(sc32[:], ps32[:])
        st_["sc32"] = sc32
        if debug:
            nc.gpsimd.dma_start(out=env["dbg_sc"].ap()[b, :, :], in_=scal[:])

        # stats matmuls: [3,512] (sum, qgx, wgx) + [1,512] (sumsq) per block
        sd = drp.tile([4, HW], F32, tag="sd", name="sd")
        for blk in range(NBLK):
            sl_ = slice(blk * BLK, (blk + 1) * BLK)
            stA = psA.tile([3, BLK], F32, tag="stA", name="stA")
            sqr = []
            for ct in range(CT):
                sq = bigp.tile([128, BLK], F32, tag="sq", name="sq")
                xs = xb[ct][:, sl_]
                nc.scalar.square(sq[:], xs[:].bitcast(F32))
                nc.tensor.matmul(stA[:], W3[:, ct, :], xs, start=(ct == 0), stop=(ct == CT - 1))
                r_ = smp.tile([1, BLK], F32, tag="sqr", name="sqr", bufs=2)
                nc.gpsimd.reduce_sum(r_[:], sq[:], axis=mybir.AxisListType.C)
                sqr.append(r_)
            eva = smp.tile([3, BLK], F32, tag="eva", name="eva", bufs=2)
            evb = smp.tile([1, BLK], F32, tag="evb", name="evb", bufs=2)
            nc.vector.tensor_copy(eva[:], stA[:])
            nc.vector.tensor_add(evb[:], sqr[0][:], sqr[1][:])
            nc.sync.dma_start(out=sd[0:3, sl_], in_=eva[:])
            nc.sync.dma_start(out=sd[3:4, sl_], in_=evb[:])
        nt = smp.tile([32, 4, 128], F32, tag="nt", name="nt")
        nc.sync.dma_start(out=nt, in_=sd[:].rearrange("s (j f) -> j s f", f=128))
        st_["nt"] = nt
        return st_

    def middle(b, st_):
        nt, sc32, xb = st_["nt"], st_["sc32"], st_["xb"]

        def st(tag):
            t = smp.tile([32, 128], F32, tag=tag, name=tag)
            return t

        m_t = st("m_t")
        nc.scalar.mul(m_t[:], nt[:, 0, :], 1.0 / C)
        v_t = st("v_t")
        nc.scalar.mul(v_t[:], nt[:, 3, :], 1.0 / C)
        msq = st("msq")
        nc.vector.tensor_mul(msq[:], m_t[:], m_t[:])
        var = st("var")
        nc.vector.tensor_sub(var[:], v_t[:], msq[:])
        stdt = st("stdt")
        nc.scalar.activation(stdt[:], var[:], AF.Sqrt, bias=eps32[:], scale=1.0)
        rstd = st("rstd")
        nc.vector.reciprocal(rstd[:], stdt[:])
        t1 = st("t1")
        nc.vector.tensor_scalar_mul(t1[:], m_t[:], sc32[:, 0:1])
        nc.vector.tensor_sub(t1[:], nt[:, 1, :], t1[:])
        nc.vector.tensor_mul(t1[:], t1[:], rstd[:])
        nc.vector.tensor_scalar_add(t1[:], t1[:], sc32[:, 1:2])
        ex_t = st("ex_t")
        zp = smp.tile([32, 1], F32, tag="zp", name="zp")
        nc.scalar.activation(ex_t[:], t1[:], AF.Exp, scale=SCALE, accum_out=zp[:])
        u_t = smp.tile([32, 128], F32R, tag="u_t", name="u_t")
        nc.vector.tensor_mul(u_t[:], ex_t[:], rstd[:])
        t4 = st("t4")
        nc.vector.tensor_scalar_mul(t4[:], m_t[:], sc32[:, 3:4])
        nc.vector.tensor_sub(t4[:], nt[:, 2, :], t4[:])
        nc.vector.tensor_mul(t4[:], t4[:], rstd[:])
        nc.vector.tensor_scalar_add(t4[:], t4[:], sc32[:, 4:5])
        ii = st("ii")
        nc.scalar.activation(ii[:], t4[:], AF.Sigmoid)
        den = st("den")
        nc.vector.tensor_scalar(out=den[:], in0=ii[:], scalar1=sc32[:, 2:3],
                                scalar2=EPS_FUSE, op0=AL.add, op1=AL.add)
        rden = st("rden")
        nc.vector.reciprocal(rden[:], den[:])
        alpha = st("alpha")
        nc.vector.tensor_mul(alpha[:], ii[:], rden[:])
        beta = st("beta")
        nc.vector.tensor_scalar_mul(beta[:], rden[:], sc32[:, 2:3])
        scrk = st("scrk")
        k1p = smp.tile([32, 1], F32, tag="k1p", name="k1p")
        nc.vector.tensor_mul(scrk[:], u_t[:].bitcast(F32), m_t[:])
        nc.vector.reduce_sum(k1p[:], scrk[:], axis=mybir.AxisListType.X)
        psz = psM.tile([1, 2], F32, tag="psm", name="psz")
        nc.tensor.matmul(psz[:, 0:1], zp[:], onescf[0:32, :], start=True, stop=True)
        nc.tensor.matmul(psz[:, 1:2], k1p[:], onescf[0:32, :], start=True, stop=True)
        zk = smp.tile([1, 4], F32, tag="zk", name="zk")
        nc.vector.tensor_copy(zk[0:1, 0:2], psz[:])
        nc.vector.reciprocal(zk[0:1, 2:3], zk[0:1, 0:1])
        zkr = smp.tile([1, 2], F32R, tag="zkr", name="zkr")
        nc.vector.tensor_copy(zkr[0:1, 0:1], zk[0:1, 2:3])
        nc.vector.tensor_copy(zkr[0:1, 1:2], zk[0:1, 1:2])
        ps128 = psM.tile([128, 2], F32, tag="psm", name="ps128")
        nc.tensor.matmul(ps128[:], onesrow[:], zkr[:], start=True, stop=True)
        sc128 = smp.tile([128, 2], F32, tag="sc128", name="sc128")
        nc.vector.tensor_copy(sc128[:], ps128[:])

        # export alpha/beta/ones/u -> DRAM -> fp32r rows
        ex4 = smp.tile([32, 4, 128], F32R, tag="ex4", name="ex4")
        nc.vector.tensor_copy(ex4[:, 0, :], alpha[:])
        nc.vector.tensor_copy(ex4[:, 1, :], beta[:])
        nc.vector.tensor_copy(ex4[:, 2, :], ones32x[:])
        nc.vector.tensor_copy(ex4[:, 3, :], u_t[:])
        ed = drp.tile([4, HW], F32R, tag="ed", name="ed")
        for s_ in range(4):
            nc.sync.dma_start(out=ed[s_:s_ + 1, :], in_=ex4[:, s_, :])
        rext = smp.tile([3, HW], F32R, tag="rext", name="rext")
        nc.sync.dma_start(out=rext, in_=ed[0:3, :])
        rext_u = smp.tile([1, HW], F32R, tag="rext_u", name="rext_u", bufs=1)
        nc.sync.dma_start(out=rext_u, in_=ed[3:4, :])
        st_["rext"] = rext
        if debug:
            nc.gpsimd.dma_start(out=env["dbg_ext"].ap()[b, :, :], in_=ed[:])

        # s-contraction: s_un[c] = sum_n u[n] x[c,n]
        s_acc = smp.tile([128, CT], F32, tag="s_acc", name="s_acc")
        for blk in range(NBLK):
            wb = psC.tile([128, BLK], F32, tag="conv", name="wb")
            nc.tensor.matmul(wb[:], onesrow[:], rext_u[:, blk * BLK:(blk + 1) * BLK],
                             start=True, stop=True)
            for ct in range(CT):
                scr = bigp.tile([128, BLK], F32, tag="scr", name="scr")
                nc.vector.tensor_mul(scr[:], xb[ct][:, blk * BLK:(blk + 1) * BLK], wb[:])
                red = smp.tile([128, 1], F32, tag="red", name="red", bufs=3)
                nc.vector.reduce_sum(red[:], scr[:], axis=mybir.AxisListType.X)
                if blk == 0:
                    nc.vector.tensor_copy(s_acc[:, ct:ct + 1], red[:])
                else:
                    nc.vector.tensor_add(s_acc[:, ct:ct + 1], s_acc[:, ct:ct + 1], red[:])
        # s = g*(s_un - K1)/Z + b
        s_sb = smp.tile([128, CT], F32R, tag="s_sb", name="s_sb")
        for ct in range(CT):
            tmp = smp.tile([128, 1], F32, tag="sfin", name="sfin")
            nc.vector.tensor_scalar(out=tmp[:], in0=s_acc[:, ct:ct + 1],
                                    scalar1=sc128[:, 1:2], scalar2=None, op0=AL.subtract)
            nc.vector.tensor_mul(tmp[:], tmp[:], g2[:, ct:ct + 1])
            nc.vector.tensor_scalar_mul(tmp[:], tmp[:], sc128[:, 0:1])
            nc.vector.tensor_add(s_sb[:, ct:ct + 1], tmp[:], bln2[:, ct:ct + 1])
        if debug:
            nc.gpsimd.dma_start(out=env["dbg_s"].ap()[b, :, :], in_=s_sb[:])

        def matvec(wmat, rhs_sb, tag):
            out_sb = smp.tile([128, CT], F32R, tag=tag, name=tag)
            for mt in range(CT):
                psm = psM.tile([128, 2], F32, tag="psm", name="psm")
                for kt in range(CT):
                    nc.tensor.matmul(psm[:], wmat[:, kt, mt * 128:(mt + 1) * 128],
                                     rhs_sb[:, kt:kt + 1].to_broadcast((128, 2)),
                                     start=(kt == 0), stop=(kt == CT - 1))
                nc.vector.tensor_copy(out_sb[:, mt:mt + 1], psm[:, 0:1])
            return out_sb

        tg_sb = matvec(wiv, s_sb, "tg_sb")
        wt_sb = matvec(wou, tg_sb, "wt_sb")
        b1_sb = matvec(wf1, wt_sb, "b1_sb")

        ext1 = smp.tile([2, CT, 128], F32R, tag="ext1", name="ext1")
        ext2 = smp.tile([4, CT, 128], F32R, tag="ext2", name="ext2")
        for mt in range(CT):
            ab1 = smp.tile([128, 2], F32R, tag="ab1", name="ab1")
            nc.vector.tensor_copy(ab1[:, 0:1], a1_sb[:, mt, b:b + 1])
            nc.vector.tensor_copy(ab1[:, 1:2], b1_sb[:, mt:mt + 1])
            pse1 = psM.tile([2, 128], F32R, tag="psm", name="pse1")
            nc.tensor.transpose(pse1[:], ab1[:], id128[:])
            nc.vector.tensor_copy(ext1[:, mt, :], pse1[:])
            ab2 = smp.tile([128, 4], F32R, tag="ab2", name="ab2")
            nc.vector.tensor_copy(ab2[:, 0:1], wv_sb[:, mt, b:b + 1])
            nc.vector.tensor_copy(ab2[:, 1:2], wt_sb[:, mt:mt + 1])
            nc.vector.tensor_copy(ab2[:, 2:3], bb2[:, mt:mt + 1])
            nc.vector.tensor_copy(ab2[:, 3:4], onescol[:])  # unused pad
            pse2 = psM.tile([4, 128], F32R, tag="psm", name="pse2")
            nc.tensor.transpose(pse2[:], ab2[:], id128[:])
            nc.vector.tensor_copy(ext2[:, mt, :], pse2[:])
        st_["ext1"], st_["ext2"] = ext1, ext2

    def pass2(b, st_):
        xb, rext, ext1, ext2 = st_["xb"], st_["rext"], st_["ext1"], st_["ext2"]
        for blk in range(NBLK):
            sl = slice(blk * BLK, (blk + 1) * BLK)
            h_ts = []
            for mt in range(CT):
                ph = psC.tile([128, BLK], F32, tag="conv", name="ph")
                nc.tensor.matmul(ph[:], ext1[:, mt, :], rext[0:2, sl], start=True, stop=False)
                for kt in range(CT):
                    nc.tensor.matmul(ph[:], wf1[:, kt, mt * 128:(mt + 1) * 128],
                                     xb[kt][:, sl], start=False, stop=(kt == CT - 1))
                h_t = bigp.tile([128, BLK], F32R, tag=f"h{mt}", name=f"h{mt}")
                nc.scalar.activation(h_t[:], ph[:], AF.Gelu, bias=cc12[:, mt:mt + 1], scale=1.0)
                h_ts.append(h_t)
            for mt in range(CT):
                po = psC.tile([128, BLK], F32, tag="conv", name="po")
                nc.tensor.matmul(po[:], ext2[0:3, mt, :], rext[0:3, sl], start=True, stop=False)
                for kt in range(CT):
                    nc.tensor.matmul(po[:], wf2[:, kt, mt * 128:(mt + 1) * 128],
                                     h_ts[kt][:], start=False, stop=(kt == CT - 1))
                ot = outp.tile([128, BLK], F32, tag="ot", name="ot")
                nc.vector.tensor_add(ot[:], po[:], xb[mt][:, sl])
                nc.sync.dma_start(out=yout.ap()[b, mt * 128:(mt + 1) * 128, sl], in_=ot[:])

    for _rep in range(REPEAT):
        sts = [pass1(b) for b in range(BSH)]
        for b in range(BSH):
            middle(b, sts[b])
        for b in range(BSH):
            pass2(b, sts[b])


def _prep_inputs(inputs):
    """Host-side weight preprocessing + per-core sharding."""
    f = lambda k: np.ascontiguousarray(np.asarray(inputs[k], dtype=np.float32))
    img = f('img_feats').reshape(B, C, HW)
    txt = f('txt_feats')
    g = f('ln_img_g'); bln = f('ln_img_b')
    w_igate = f('w_igate')[0]
    v2 = lambda v: np.ascontiguousarray(v.reshape(CT, 128).T)  # [C] -> [128, 2]
    common = {
        'wf1T': np.ascontiguousarray(f('w_ffn1').T),
        'wf2T': np.ascontiguousarray(f('w_ffn2').T),
        'woutT': np.ascontiguousarray(f('w_out').T),
        'wivT': np.ascontiguousarray(f('w_img_v').T),
        'wik': f('w_img_k'),
        'wtqT': np.ascontiguousarray(f('w_txt_q').T),
        'wtvT': np.ascontiguousarray(f('w_txt_v').T),
        'g2d': v2(g),
        'bln2d': v2(bln),
        'bb2d': v2(f('b_out') + f('b_ffn2')),
        'cc12d': v2(f('w_ffn1') @ f('b_out') + f('b_ffn1')),
        'wg2d': v2(w_igate * g),
        'gt2d': f('ln_txt_g').reshape(1, GUIDE),
        'bt2d': f('ln_txt_b').reshape(1, GUIDE),
        'wtg2d': f('w_tgate').reshape(1, GUIDE),
        'btg2d': np.full((2, 1), f('b_igate')[0] * 0 + f('b_tgate')[0], np.float32),
        'misc': np.concatenate([
            np.array([np.sum(w_igate * g), np.dot(w_igate, bln) + f('b_igate')[0]],
                     np.float32), np.zeros(6, np.float32)]).reshape(1, 8),
        'conesr': np.ones((1, 128), np.float32),
        'id128d': np.eye(128, dtype=np.float32),
    }
    in_maps = []
    for core in range(NCORES):
        sl = slice(core * BSH, (core + 1) * BSH)
        m = dict(common)
        m['img'] = np.ascontiguousarray(img[sl])
        m['txt'] = np.ascontiguousarray(txt[sl])
        in_maps.append(m)
    return in_maps


def get_nc(debug=False):
    key = ('dbg' if debug else 'rel')
    if key not in _CACHE:
        _CACHE[key] = _build(debug)
    return _CACHE[key]


def run(inputs, debug=False):
    nc = get_nc(debug)
    in_maps = _prep_inputs(inputs)
    res = bass_utils.run_bass_kernel_spmd(nc, in_maps, core_ids=list(range(NCORES)))
    return res


def kernel(**inputs):
    res = run(inputs)
    out = np.empty((B, C, HW), np.float32)
    for core in range(NCORES):
        out[core * BSH:(core + 1) * BSH] = res.results[core]['yout']
    return out.reshape(B, C, H, W)

